# revision 5
# baseline (speedup 1.0000x reference)
"""LSTM (T=4096, B=2048, I=1, H=4) + linear head, on 8 trn2 NeuronCores.

v4: 2 interleaved groups x 4 col-stacked streams (1024-col tiles).  All
elementwise work on DVE (GPSIMD compute removed: its SBUF traffic stole DVE
bandwidth); fc output projection writes into the gate-PSUM's dead rows 0:8
(free after ACT1 consumes them) and is DMA'd PSUM->DRAM directly, which
eliminates the fc PSUM tile, the SBUF stage buffers and the scalar-engine
copies.  y returned fp32 from PSUM.
"""

import numpy as np

T, B, I, H = 4096, 2048, 1, 4
NCORES = 8
NGRP = 2
SPG = 4
S = NGRP * SPG
CHUNK = T // (NCORES * S)   # 64
WARM = 18
NT = CHUNK + WARM           # 82
SLICES = 8
SCOL = 256
COLS = SPG * SCOL           # 1024
HALF = COLS // 2
RING = 16
XCH = 4
assert RING % XCH == 0
FCG = 4
NGROUPS = NT // FCG + 1
YPAD = NGROUPS * FCG

GATE_SCALE = (0.5, 0.5, 0.5, 1.0)   # blocks [f, o, i, g]
REF_ROW = (4, 12, 0, 8)


def _prep_weights(w_ih, w_hh, b_ih, b_hh, w_fc, b_fc):
    bias = (b_ih + b_hh).astype(np.float64)
    wblk = np.zeros((48, 128), np.float64)
    wfc = np.zeros((48, 8), np.float64)
    for s in range(SLICES):
        for blk in range(4):
            sc = GATE_SCALE[blk]
            for j in range(4):
                row = REF_ROW[blk] + j
                m = blk * 32 + j * 8 + s
                for c in range(4):
                    wblk[c * 8 + s, m] = w_hh[row, c] * sc
                wblk[32 + s, m] = bias[row] * sc
                wblk[40 + s, m] = w_ih[row, 0] * sc
        for c in range(4):
            wfc[c * 8 + s, s] = w_fc[0, c]
        wfc[32 + s, s] = b_fc[0]
    return wblk.astype(np.float16), wfc.astype(np.float16)


def _build_program():
    from contextlib import ExitStack
    import concourse.bass as bass
    from concourse import mybir

    fp32 = mybir.dt.float32
    fp16 = mybir.dt.float16
    TT = mybir.AluOpType
    Act = mybir.ActivationFunctionType

    nc = bass.Bass("TRN2", target_bir_lowering=False, debug=False,
                   num_devices=NCORES)
    xcd = nc.dram_tensor("xc", [NGRP, SLICES, NT, COLS], fp16,
                         kind="ExternalInput")
    wblkd = nc.dram_tensor("wblk", [48, 128], fp16, kind="ExternalInput")
    wfcd = nc.dram_tensor("wfc", [48, 8], fp16, kind="ExternalInput")
    onesd = nc.dram_tensor("ones", [8, RING, COLS], fp16,
                           kind="ExternalInput")
    ycd = nc.dram_tensor("yc", [NGRP, NGROUPS, 128, COLS], fp16,
                         kind="ExternalOutput")

    NCHUNK = (NT + XCH - 1) // XCH

    with ExitStack() as ctx:
        ec = ctx.enter_context
        block = ec(nc.Block())
        sem = {}
        for g in range(NGRP):
            for name in ("pe", "pe2", "acts", "dvec", "dveh",
                         "xsem", "init", "osem", "copy"):
                sem[g, name] = ec(nc.semaphore(f"{name}{g}"))
        wsem = ec(nc.semaphore("wsem"))

        sring, tgS, sigX, cF, tctF, igb, fcb, stage = ({} for _ in range(8))
        gates, fcps = {}, {}
        for g in range(NGRP):
            sring[g] = ec(nc.sbuf_tensor(f"sring{g}", [48, RING, COLS], fp16))
            tgS[g] = ec(nc.sbuf_tensor(f"tgS{g}", [128, COLS], fp16))
            sigX[g] = ec(nc.sbuf_tensor(f"sigX{g}", [128, COLS], fp16))
            cF[g] = ec(nc.sbuf_tensor(f"cF{g}", [32, COLS], fp16))
            tctF[g] = ec(nc.sbuf_tensor(f"tctF{g}", [64, COLS], fp16))
            igb[g] = ec(nc.sbuf_tensor(f"igb{g}", [32, COLS], fp16))
            fcb[g] = ec(nc.sbuf_tensor(f"fcb{g}", [32, COLS], fp16))
            stage[g] = [ec(nc.sbuf_tensor(f"stage{g}_{i}", [128, COLS], fp16))
                        for i in range(2)]
            gates[g] = ec(nc.psum_tensor(f"gates{g}", [128, COLS], fp32))
            fcps[g] = ec(nc.psum_tensor(f"fcps{g}", [128, COLS], fp32))
        wblk = ec(nc.sbuf_tensor("wblk_sb", [48, 128], fp16))
        wfc = ec(nc.sbuf_tensor("wfc_sb", [48, 8], fp16))

        @block.sync
        def _(sp):
            sp.dma_start(wblk.ap(), wblkd.ap()).then_inc(wsem, 16)
            sp.dma_start(wfc.ap(), wfcd.ap()).then_inc(wsem, 16)
            for g in range(NGRP):
                sp.dma_start(sring[g].ap()[32:40, :, :],
                             onesd.ap()).then_inc(wsem, 16)
            for k in range(NCHUNK):
                w = min(XCH, NT - k * XCH)
                for g in range(NGRP):
                    if k >= RING // XCH:
                        sp.wait_ge(sem[g, "pe2"], k * XCH - RING + XCH)
                    slot = (k * XCH) % RING
                    sp.dma_start(
                        sring[g].ap()[40:48, slot:slot + w, :],
                        xcd.ap()[g][:, k * XCH:k * XCH + w, :],
                    ).then_inc(sem[g, "xsem"], 16)

        @block.tensor
        def _(pe):
            pe.wait_ge(wsem, 64)
            for g in range(NGRP):
                pe.wait_ge(sem[g, "init"], 1)
            for t in range(NT):
                for g in range(NGRP):
                    slot = t % RING
                    if t % XCH == 0:
                        pe.wait_ge(sem[g, "xsem"], 16 * (t // XCH + 1))
                    if t > 0:
                        pe.wait_ge(sem[g, "dveh"], 2 * t - 1)
                    pe.matmul(gates[g].ap()[:, 0:HALF], wblk.ap(),
                              sring[g].ap()[:, slot, 0:HALF],
                              start=True, stop=True).then_inc(sem[g, "pe"], 1)
                    if t > 0:
                        pe.wait_ge(sem[g, "dveh"], 2 * t)
                    pe.matmul(gates[g].ap()[:, HALF:COLS], wblk.ap(),
                              sring[g].ap()[:, slot, HALF:COLS],
                              start=True, stop=True).then_inc(sem[g, "pe"], 1)
                if t > 0:
                    tau = t - 1
                    for g in range(NGRP):
                        slot = tau % RING
                        q = tau % FCG
                        if q == 0 and tau >= FCG:
                            pe.wait_ge(sem[g, "copy"], tau // FCG)
                        pe.matmul(fcps[g].ap()[32 * q:32 * q + 8, 0:HALF],
                                  wfc.ap(), sring[g].ap()[:, slot, 0:HALF],
                                  start=True, stop=True,
                                  tile_position=(0, 32 * q))
                        pe.matmul(fcps[g].ap()[32 * q:32 * q + 8, HALF:COLS],
                                  wfc.ap(), sring[g].ap()[:, slot, HALF:COLS],
                                  start=True, stop=True,
                                  tile_position=(0, 32 * q)
                                  ).then_inc(sem[g, "pe2"], 1)
            tau = NT - 1
            for g in range(NGRP):
                slot = tau % RING
                q = tau % FCG
                pe.matmul(fcps[g].ap()[32 * q:32 * q + 8, 0:HALF],
                          wfc.ap(), sring[g].ap()[:, slot, 0:HALF],
                          start=True, stop=True, tile_position=(0, 32 * q))
                pe.matmul(fcps[g].ap()[32 * q:32 * q + 8, HALF:COLS],
                          wfc.ap(), sring[g].ap()[:, slot, HALF:COLS],
                          start=True, stop=True, tile_position=(0, 32 * q)
                          ).then_inc(sem[g, "pe2"], 1)
            # final fc: slot NT -> y row NT at q = NT % FCG
            for g in range(NGRP):
                pe.wait_ge(sem[g, "dveh"], 2 * NT)
                q = NT % FCG
                slot = NT % RING
                pe.matmul(fcps[g].ap()[32 * q:32 * q + 8, 0:HALF],
                          wfc.ap(), sring[g].ap()[:, slot, 0:HALF],
                          start=True, stop=True, tile_position=(0, 32 * q))
                pe.matmul(fcps[g].ap()[32 * q:32 * q + 8, HALF:COLS],
                          wfc.ap(), sring[g].ap()[:, slot, HALF:COLS],
                          start=True, stop=True, tile_position=(0, 32 * q)
                          ).then_inc(sem[g, "pe2"], 1)

        @block.scalar
        def _(act):
            for t in range(NT):
                for g in range(NGRP):
                    act.wait_ge(sem[g, "pe"], 2 * t + 1)
                    act.activation(tgS[g].ap()[:, 0:HALF],
                                   gates[g].ap()[:, 0:HALF],
                                   Act.Tanh).then_inc(sem[g, "acts"], 1)
                    act.wait_ge(sem[g, "pe"], 2 * t + 2)
                    act.activation(tgS[g].ap()[:, HALF:COLS],
                                   gates[g].ap()[:, HALF:COLS],
                                   Act.Tanh).then_inc(sem[g, "acts"], 1)
                if t % FCG == 0 and t >= FCG:
                    G = t // FCG - 1
                    for g in range(NGRP):
                        act.wait_ge(sem[g, "pe2"], 4 * G + 4)
                        if G >= 2:
                            act.wait_ge(sem[g, "osem"], 16 * (G - 1))
                        act.activation(stage[g][G % 2].ap(), fcps[g].ap(),
                                       Act.Copy).then_inc(sem[g, "copy"], 1)
                for g in range(NGRP):
                    act.wait_ge(sem[g, "dvec"], t + 1)
                    act.activation(tctF[g].ap()[32:64, 0:HALF],
                                   cF[g].ap()[:, 0:HALF],
                                   Act.Tanh).then_inc(sem[g, "acts"], 1)
                    act.activation(tctF[g].ap()[32:64, HALF:COLS],
                                   cF[g].ap()[:, HALF:COLS],
                                   Act.Tanh).then_inc(sem[g, "acts"], 1)
            for g in range(NGRP):
                G = NT // FCG
                act.wait_ge(sem[g, "pe2"], NT + 1)
                act.wait_ge(sem[g, "osem"], 16 * (G - 1))
                act.activation(stage[g][G % 2].ap(), fcps[g].ap(),
                               Act.Copy).then_inc(sem[g, "copy"], 1)

        @block.vector
        def _(dve):
            for g in range(NGRP):
                dve.memset(sring[g].ap()[0:32, 0, :], 0.0)
                dve.memset(cF[g].ap(), 0.0)
                dve.memset(fcps[g].ap(), 0.0).then_inc(sem[g, "init"], 1)
            for t in range(NT):
                for g in range(NGRP):
                    dve.wait_ge(sem[g, "acts"], 4 * t + 2)
                    dve.tensor_scalar(sigX[g].ap()[0:64], tgS[g].ap()[0:64],
                                      0.5, 0.5, TT.mult, TT.add)
                    dve.tensor_scalar(sigX[g].ap()[96:128], tgS[g].ap()[64:96],
                                      0.5, 0.5, TT.mult, TT.add)
                    dve.tensor_tensor(igb[g].ap(), sigX[g].ap()[96:128],
                                      tgS[g].ap()[96:128], TT.mult)
                    dve.tensor_tensor(fcb[g].ap(), sigX[g].ap()[0:32],
                                      cF[g].ap(), TT.mult)
                    dve.tensor_tensor(cF[g].ap(), igb[g].ap(),
                                      fcb[g].ap(), TT.add
                                      ).then_inc(sem[g, "dvec"], 1)
                for g in range(NGRP):
                    dve.wait_ge(sem[g, "acts"], 4 * t + 3)
                    dve.tensor_tensor(
                        sring[g].ap()[0:32, (t + 1) % RING, 0:HALF],
                        sigX[g].ap()[32:64, 0:HALF],
                        tctF[g].ap()[32:64, 0:HALF],
                        TT.mult).then_inc(sem[g, "dveh"], 1)
                    dve.wait_ge(sem[g, "acts"], 4 * t + 4)
                    dve.tensor_tensor(
                        sring[g].ap()[0:32, (t + 1) % RING, HALF:COLS],
                        sigX[g].ap()[32:64, HALF:COLS],
                        tctF[g].ap()[32:64, HALF:COLS],
                        TT.mult).then_inc(sem[g, "dveh"], 1)

        @block.gpsimd
        def _(gp):
            for G in range(NGROUPS):
                for g in range(NGRP):
                    gp.wait_ge(sem[g, "copy"], G + 1)
                    gp.dma_start(
                        ycd.ap()[g, G],
                        stage[g][G % 2].ap(),
                    ).then_inc(sem[g, "osem"], 16)

    return nc


def kernel(**inputs):
    from concourse.bass_utils import run_bass_kernel_spmd

    x = np.ascontiguousarray(
        np.asarray(inputs["x"], np.float32).reshape(T, B)).astype(np.float16)
    wblk, wfc = _prep_weights(
        np.asarray(inputs["w_ih"], np.float32), np.asarray(inputs["w_hh"], np.float32),
        np.asarray(inputs["b_ih"], np.float32), np.asarray(inputs["b_hh"], np.float32),
        np.asarray(inputs["w_fc"], np.float32), np.asarray(inputs["b_fc"], np.float32))

    nc = _build_program()
    in_maps = []
    for core in range(NCORES):
        xc = np.zeros((NGRP, SLICES, NT, COLS), np.float16)
        for g in range(NGRP):
            for st in range(SPG):
                sidx = g * SPG + st
                g0 = max((core * S + sidx) * CHUNK - WARM, 0)
                xs = x[g0:g0 + NT].reshape(NT, SLICES, SCOL)
                xc[g, :, :, st * SCOL:(st + 1) * SCOL] = xs.transpose(1, 0, 2)
        in_maps.append({"xc": xc, "wblk": wblk, "wfc": wfc,
                        "ones": np.ones((8, RING, COLS), np.float16)})

    res = run_bass_kernel_spmd(nc, in_maps, core_ids=list(range(NCORES)))

    y = np.empty((T, B), np.float32)
    for core in range(NCORES):
        # yc[g, G, 32q+s, st*SCOL+c] = y-row (G*FCG+q) of stream g*SPG+st
        yc = res.results[core]["yc"].reshape(
            NGRP, NGROUPS, FCG, 32, SPG, SCOL)[:, :, :, 0:SLICES]
        yc = yc.reshape(NGRP, YPAD, SLICES, SPG, SCOL).transpose(0, 3, 1, 2, 4)
        for g in range(NGRP):
            for st in range(SPG):
                sidx = g * SPG + st
                out0 = (core * S + sidx) * CHUNK
                g0 = max(out0 - WARM, 0)
                r0 = out0 - g0 + 1
                y[out0:out0 + CHUNK] = yc[g, st, r0:r0 + CHUNK].reshape(
                    CHUNK, B).astype(np.float32)
    return y.reshape(T, B, 1)
